# revision 45
# baseline (speedup 1.0000x reference)
"""Trainium2 Bass kernel for nn_CrossAttentionFusion.

Math: softmax over kv_len==1 is identically 1.0, so the attention output is
v broadcast over the N (patch) axis and the whole module reduces to

    out[b, n, :] = cnn[b] @ (Wkv[:, C:] @ Wp) + bp        (independent of n)

W_eff = Wkv[:, C:] @ Wp is a weight-only constant, folded on the host.

Strategy: COLUMN-parallel over the C=768 output columns across 8 NeuronCores
(96 columns per core, full batch on every core), fp16 end-to-end on device.
Per core the inputs are tiny (~0.75 MB fp16) and the output write dominates:
64*576*96 fp16 = 7.08 MB. The harness gate is rel_err < 2e-2; fp16 adds ~4e-4.

Pipeline per core:
 1. Three input DMAs (weff+bias fused, cnnT+ones fused, sel one-hots).
 2. 17 accumulating matmuls -> ps_row[64, 96] = cnn @ W_eff + bp (bias is a
    17th contraction chunk: ones-row in cnnT x bp-row in weff).
 3. One cast copy + one stride-0 broadcast copy -> row_rep[64, 864] fp16
    (row replicated 9x along free axis).
 4. Output groups of ascending batch count [4, 4, 8, 16, 32]: a one-hot
    matmul fans GB rows out to 128 partitions (PPB partitions per batch),
    a DVE copy casts PSUM->SBUF fp16, then one DMA per group writes the
    batch-contiguous rows with 1728-B descriptors (stride-0-source j
    broadcast doubles each partition's 9 SBUF rows to its full dst rows).
    Small groups first so the DMA stream starts ~3 us earlier; the last
    group is split across both HWDGE rings (j-slices) for ring balance.
"""

import sys

sys.path.insert(0, "/opt/trn_rl_repo")

import numpy as np

import concourse.bass as bass
import concourse.mybir as mybir
from concourse import bacc
from concourse.bass_utils import run_bass_kernel_spmd
from concourse.tile import TileContext

F32 = mybir.dt.float32
F16 = mybir.dt.float16

NCORES = 8
B, N, C, CNN = 64, 576, 768, 2048
CPC = C // NCORES  # 96 output columns per core
KC = CNN // 128 + 1  # 16 contraction chunks + 1 bias chunk
# output groups: (batch count, DMA replication, matmul replication).
# REP_dma sets the descriptor size (REP*96*2 bytes): bigger descriptors
# amortize per-packet SDMA overhead (the slow engine 15 especially). When
# REP_dma > REP_mm the bc tile is doubled by a cheap SBUF->SBUF copy
# instead of extra fan-out matmuls (the replicated content repeats).
# Small starter groups let the stream begin before the big fan-outs run.
GROUPS = [(8, 9, 9), (8, 18, 9), (16, 18, 9), (32, 36, 9)]
NG = len(GROUPS)
MAXREP = 18  # max REP_mm: row_rep replication depth
MAXDMA = 36  # max REP_dma: bc tile width


def _build_bass():
    nc = bacc.Bacc(None, target_bir_lowering=False, debug=False, num_devices=NCORES)

    x_cnnT = nc.declare_dram_parameter("cnnT", [128, KC * B], F16, isOutput=False)
    x_weff = nc.declare_dram_parameter("weff", [128, KC * CPC], F16, isOutput=False)
    x_sel = nc.declare_dram_parameter("sel", [B, NG * 128], F16, isOutput=False)
    yo = nc.declare_dram_parameter("out", [B * N, CPC], F16, isOutput=True)

    with TileContext(nc) as tc:
        with (
            tc.tile_pool(name="singles", bufs=1) as singles,
            tc.tile_pool(name="psum_bc", bufs=2, space="PSUM") as psum_bc,
            tc.tile_pool(name="bc_sb", bufs=3) as bc_sb,
        ):
            # staged loads: halves overlap the row matmuls; the tiny third
            # chunk (last k + bias) completes early so the final matmuls
            # aren't gated on the bulk transfer's completion semaphore
            weff_t = singles.tile([128, KC * CPC], F16, tag="weff")
            for lo, hi in ((0, 8), (8, 16), (16, KC)):
                nc.sync.dma_start(
                    out=weff_t[:, lo * CPC : hi * CPC],
                    in_=x_weff[:, lo * CPC : hi * CPC],
                )
            cnnT_t = singles.tile([128, KC * B], F16, tag="cnnT")
            for lo, hi in ((0, 8), (8, 16), (16, KC)):
                nc.scalar.dma_start(
                    out=cnnT_t[:, lo * B : hi * B],
                    in_=x_cnnT[:, lo * B : hi * B],
                )
            sel_t = singles.tile([B, NG * 128], F16, tag="sel")
            nc.scalar.dma_start(out=sel_t[:], in_=x_sel[:, :])

            row_rep = singles.tile([B, MAXREP * CPC], F16, tag="row_rep")

            # Projection (+bias via the 17th chunk): row = cnn @ W_eff + bp.
            # ps_row borrows a rotating psum_bc buffer (row phase finishes
            # before the group fan-outs cycle back to it).
            ps_row = psum_bc.tile([128, 2048], F32, tag="ps_bc", name="ps_row")
            for kc in range(KC):
                nc.tensor.matmul(
                    ps_row[0:B, 0:CPC],
                    cnnT_t[:, kc * B : (kc + 1) * B],
                    weff_t[:, kc * CPC : (kc + 1) * CPC],
                    start=(kc == 0),
                    stop=(kc == KC - 1),
                )
            # PSUM->SBUF cast, then two SBUF-source stride-0 broadcast copies
            # (PSUM-sourced broadcast reads are ~3x slower). The first small
            # copy unblocks group 0 (REP=3); the second fills the rest while
            # group 0's fan-out matmul runs.
            nc.vector.tensor_copy(row_rep[:, 0:CPC], ps_row[0:B, 0:CPC])
            # group 0 needs 9 reps now; reps 9-18 (only needed by the last
            # group's fan-out matmuls) are filled after group 0 is issued
            nc.vector.tensor_copy(
                row_rep[:, CPC : 9 * CPC].rearrange("b (r c) -> b r c", r=8),
                row_rep[:, 0:CPC].unsqueeze(1).broadcast_to((B, 8, CPC)),
            )

            # Output groups.
            b0 = 0
            for gi, (GB, REP, RMM) in enumerate(GROUPS):
                rows_g = GB * N  # dram rows this group
                jp = rows_g // 128  # dst rows per partition
                jb = jp // REP  # stride-0 j repeats in the DMA
                frep = REP * CPC  # bc tile elems per partition
                fmm = RMM * CPC  # elems produced by the fan-out matmuls
                nch = min(fmm, 432)  # PSUM chunk (<=512 fp32 per bank)
                ps_bc = psum_bc.tile([128, 2048], F32, tag="ps_bc", name="ps_bc")
                bc_t = bc_sb.tile([128, MAXDMA * CPC], F16, tag="bc_t", name="bc_t")
                for s in range(fmm // nch):
                    nc.tensor.matmul(
                        ps_bc[:, s * 512 : s * 512 + nch],
                        sel_t[:, gi * 128 : (gi + 1) * 128],
                        row_rep[:, s * nch : (s + 1) * nch],
                        start=True,
                        stop=True,
                    )
                    if s % 2 == 0:
                        nc.vector.tensor_copy(
                            bc_t[:, s * nch : (s + 1) * nch],
                            ps_bc[:, s * 512 : s * 512 + nch],
                        )
                    else:
                        nc.scalar.copy(
                            bc_t[:, s * nch : (s + 1) * nch],
                            ps_bc[:, s * 512 : s * 512 + nch],
                        )
                w = fmm
                while w < frep:  # double the tile: replicated content repeats
                    nc.vector.tensor_copy(bc_t[:, w : 2 * w], bc_t[:, 0:w])
                    w *= 2
                dst = yo[b0 * N : b0 * N + rows_g, :].rearrange(
                    "(p j r) c -> p j (r c)", p=128, j=jb, r=REP
                )
                src = bc_t[:, 0:frep].unsqueeze(1).broadcast_to((128, jb, frep))
                if gi == NG - 1:
                    # split the last group across the SWDGE queue + a HWDGE
                    # ring: the third queue's packets interleave with the
                    # previous group's tail instead of queuing behind it
                    h = jb // 2
                    nc.gpsimd.dma_start(out=dst[:, 0:h, :], in_=src[:, 0:h, :])
                    nc.scalar.dma_start(out=dst[:, h:jb, :], in_=src[:, h:jb, :])
                else:
                    # alternate rings so neither sequencer's dma_start issue
                    # cost (~0.7 us each) throttles the early stream cadence
                    eng = nc.scalar if gi % 2 == 1 else nc.sync
                    eng.dma_start(out=dst, in_=src)
                if gi == 0:
                    nc.vector.tensor_copy(
                        row_rep[:, 9 * CPC :].rearrange("b (r c) -> b r c", r=9),
                        row_rep[:, 0:CPC].unsqueeze(1).broadcast_to((B, 9, CPC)),
                    )
                b0 += GB

    nc.compile()
    return nc


_NC = None


def _get_nc():
    global _NC
    if _NC is None:
        _NC = _build_bass()
    return _NC


def _prepare_in_maps(image_patches, cnn_feature_vector, Wq, Wkv, Wp, bp):
    Weff = np.ascontiguousarray(Wkv[:, C:]) @ Wp  # (2048, 768) fp32
    # contraction chunks: 16 x 128 rows of cnn/W_eff + 1 bias chunk
    cnnT = np.zeros((128, KC * B), dtype=np.float16)
    cnnT[:, : (KC - 1) * B] = (
        cnn_feature_vector.T.reshape(KC - 1, 128, B)
        .transpose(1, 0, 2)
        .reshape(128, (KC - 1) * B)
    )
    cnnT[0, (KC - 1) * B :] = 1.0  # ones row: picks up the bias chunk

    # sel[b, gi*128 + p] = 1 iff b == b0_gi + p // (128 // GB_gi)
    sel = np.zeros((B, NG * 128), dtype=np.float16)
    b0 = 0
    for gi, (GB, REP, RMM) in enumerate(GROUPS):
        ppb = 128 // GB
        for k in range(GB):
            sel[b0 + k, gi * 128 + k * ppb : gi * 128 + (k + 1) * ppb] = 1.0
        b0 += GB

    in_maps = []
    for core in range(NCORES):
        c0 = core * CPC
        weff = np.zeros((128, KC * CPC), dtype=np.float16)
        weff[:, : (KC - 1) * CPC] = (
            Weff[:, c0 : c0 + CPC]
            .reshape(KC - 1, 128, CPC)
            .transpose(1, 0, 2)
            .reshape(128, (KC - 1) * CPC)
        )
        weff[0, (KC - 1) * CPC :] = bp[c0 : c0 + CPC]  # bias chunk
        in_maps.append({"cnnT": cnnT, "weff": weff, "sel": sel})
    return in_maps


def _assemble(res):
    out = np.empty((B, N, C), dtype=np.float32)
    for i in range(NCORES):
        out[:, :, i * CPC : (i + 1) * CPC] = res.results[i]["out"].reshape(B, N, CPC)
    return out


def kernel(**inputs) -> np.ndarray:
    inputs = {k: np.asarray(v) for k, v in inputs.items()}
    nc = _get_nc()
    in_maps = _prepare_in_maps(**inputs)
    res = run_bass_kernel_spmd(nc, in_maps, core_ids=list(range(NCORES)))
    return _assemble(res)


def kernel_traced(**inputs):
    """kernel() + HW profile; returns (output, BassKernelResults)."""
    inputs = {k: np.asarray(v) for k, v in inputs.items()}
    nc = _get_nc()
    in_maps = _prepare_in_maps(**inputs)
    res = run_bass_kernel_spmd(
        nc, in_maps, core_ids=list(range(NCORES)), trace=True
    )
    return _assemble(res), res
